# revision 48
# baseline (speedup 1.0000x reference)
"""GCN (2-layer GraphConv + edge scorer) on 8 Trainium2 NeuronCores — v3.

Strategy (dst-sharded, Pool desc-gen minimized):
  - Nodes padded to 50176 = 8 x 49 x 128; core i owns dst nodes
    [i*6272, (i+1)*6272). Edges sorted (dst block, src half); per-(block,
    half) groups padded to whole 128-edge tiles (counts = max over cores so
    one SPMD program fits all). Pad slots carry src16 = -1: the Q7 SWDGE
    desc-gen trims trailing negative indices, so padding costs nothing.
  - ONE dma_gather per (block, half) group (MAX_GT >= T), cycling the 4
    SWDGE queues so one call's ring drain overlaps the next call's
    descriptor generation on the Pool engine (the kernel bottleneck).
  - Normalization split: rsq_out folded into X on the host, rsq_in applied
    post-aggregation per dst block. One-hot scatter tiles are pure 0/1,
    host-precomputed in bf16 (TensorE fast path) and streamed.
  - h1 = (X*rsq_out)@W1 computed per owned block, AllGathered in bf16;
    TensorE scatter-accumulates aggT per block in PSUM via one-hot matmul.
  - Layer 2 gathers h2 = (x1*rsq_out)@W2 rows (128-wide bf16), so gather
    traffic and scatter matmuls are halved vs gathering x1.
  - Scores: s_src/s_dst per node from x2T via one matmul; s_src replicated
    into a [node,128]-bf16 table (tensor_scalar x ones), AllGathered,
    fetched per edge with a transpose-mode gather (edges land on the free
    dim); s_dst expanded per edge locally via matmul with host-streamed
    transposed one-hot tiles. sigmoid(+bp) on ScalarE.
Host does index-only preprocessing and reassembles per-core score slots
into the original edge order.
"""
import os
import sys

_REPO = os.environ.get("TRN_RL_REPO", "/opt/trn_rl_repo")
if _REPO not in sys.path:
    sys.path.insert(0, _REPO)

import ml_dtypes
import numpy as np

import concourse.bacc as bacc
import concourse.tile as tile
from concourse import mybir
from concourse.bass_utils import run_bass_kernel_spmd

P = 128
NCORES = 8
N_NODES = 50000
NPAD = 50176            # 8 * 49 * 128
BPC = NPAD // NCORES // P   # blocks per core = 49
HALF = NPAD // 2        # int16 gather index limit -> lo/hi table split
IN_F = 256
HID = 256
OUT_F = 128

f32 = mybir.dt.float32
bf16 = mybir.dt.bfloat16
i16 = mybir.dt.int16
fp8 = mybir.dt.float8e4
IND_BF16 = os.environ.get("GCN2_IND_BF16", "1") == "1"
ind_dt = bf16 if IND_BF16 else fp8
D_FALLBACK = os.environ.get("GCN2_D_FALLBACK", "0") == "1"
NQ = int(os.environ.get("GCN2_NQ", "4"))       # SWDGE queues to cycle
SCRATCH = int(os.environ.get("GCN2_SCRATCH", "16384"))


def _wrap_idx(idx_flat):
    """dma_gather index layout: idx k -> [k%16, k//16], replicated 8x."""
    n = idx_flat.shape[0]
    w = idx_flat.reshape(n // 16, 16).T
    return np.tile(w, (8, 1)).astype(np.int16)


def build_program(T_arr, trace_label="gcn2"):
    """One SPMD program for all cores. T_arr: [BPC, 2] per-(block, half)
    tile counts (shared across cores)."""
    NB = T_arr.shape[0]
    TE = int(T_arr.sum())                 # total edge tiles per core
    NL = NB * P                           # nodes per core
    # tiles per gather call. Hard cap: 1024 indices (8 tiles) — the SWDGE
    # descriptor ring holds ~65 descs/engine and a single bigger call
    # deadlocks in await_space. Default 5 so a 9-tile group splits 5+4 and
    # BOTH calls fit the ring outright: descriptor-gen then never throttles
    # to the ring-drain rate mid-call (measured 2.33ms -> 2.07ms).
    MAX_GT = int(os.environ.get("GCN2_MAXGT", str(min(5, int(T_arr.max())))))
    PHASE = int(os.environ.get("GCN2_PHASE", "4"))
    AG_MID = int(os.environ.get("GCN2_AG_MID", "25"))  # blocks in chunk A
    AG_OVL = os.environ.get("GCN2_AG_OVL", "1") == "1"  # fire chunk A mid-phase

    nc = bacc.Bacc("TRN2", target_bir_lowering=False, debug=False,
                   enable_asserts=True, num_devices=NCORES,
                   num_swdge_queues=NQ,
                   dynamic_dma_scratch_size=SCRATCH)

    xT_d = nc.dram_tensor("xT", [P, NB * 2 * P], bf16, kind="ExternalInput")
    w1_d = nc.dram_tensor("w1", [P, 2 * HID], bf16, kind="ExternalInput")
    w2_d = nc.dram_tensor("w2", [P, 2 * OUT_F], bf16, kind="ExternalInput")
    wp_d = nc.dram_tensor("wp12", [P, 2], f32, kind="ExternalInput")
    b1_d = nc.dram_tensor("b1c", [P, 2], f32, kind="ExternalInput")
    b2_d = nc.dram_tensor("b2c", [P, 1], f32, kind="ExternalInput")
    bp_d = nc.dram_tensor("bp", [P, 1], f32, kind="ExternalInput")
    rsqi_d = nc.dram_tensor("rsqi", [P, NL], f32, kind="ExternalInput")
    rsqo_d = nc.dram_tensor("rsqo", [P, NB], f32, kind="ExternalInput")
    src16_d = nc.dram_tensor("src16", [P, 8 * TE], i16, kind="ExternalInput")
    ind_d = nc.dram_tensor("ind8", [P, TE * P], ind_dt, kind="ExternalInput")
    indT_d = nc.dram_tensor("indT8", [P, TE * P], ind_dt, kind="ExternalInput")
    if D_FALLBACK:
        dst16_d = nc.dram_tensor("dst16", [P, 8 * TE], i16,
                                 kind="ExternalInput")
    scoresPT_d = nc.dram_tensor("scoresPT", [P, TE], f32,
                                kind="ExternalOutput")
    TBMAX = int((T_arr[:, 0] + T_arr[:, 1]).max())

    with tile.TileContext(nc) as tc:
        with (
            tc.tile_pool(name="cons", bufs=1) as cons,
            tc.tile_pool(name="sb", bufs=3) as sb,
            tc.tile_pool(name="gp", bufs=4) as gp,
            tc.tile_pool(name="ps", bufs=3, space="PSUM") as ps,
            tc.tile_pool(name="psmm", bufs=2, space="PSUM") as psmm,
            tc.tile_pool(name="dram", bufs=1, space="DRAM") as dr,
        ):
            # ---- resident constants ----
            w1_sb = cons.tile([P, 2 * HID], bf16, name="w1_sb")
            w2_sb = cons.tile([P, 2 * OUT_F], bf16, name="w2_sb")
            wp_sb = cons.tile([P, 2], f32, name="wp_sb")
            b1_sb = cons.tile([P, 2], f32, name="b1_sb")
            b2_sb = cons.tile([P, 1], f32, name="b2_sb")
            bp_sb = cons.tile([P, 1], f32, name="bp_sb")
            rsqi_sb = cons.tile([P, NL], f32, name="rsqi_sb")
            rsqo_sb = cons.tile([P, NB], f32, name="rsqo_sb")
            src16 = cons.tile([P, 8 * TE], i16, name="src16")
            if D_FALLBACK:
                dst16 = cons.tile([P, 8 * TE], i16, name="dst16")
                nc.sync.dma_start(dst16[:], dst16_d[:])
            sdcol = cons.tile([P, NB], bf16, name="sdcol")
            ones_sb = cons.tile([P, P], bf16, name="ones_sb")
            nc.vector.memset(ones_sb[:], 1.0)
            for s_t, d_t in [(w1_sb, w1_d), (w2_sb, w2_d), (wp_sb, wp_d),
                             (b1_sb, b1_d), (b2_sb, b2_d), (bp_sb, bp_d),
                             (rsqi_sb, rsqi_d), (rsqo_sb, rsqo_d),
                             (src16, src16_d)]:
                nc.sync.dma_start(s_t[:], d_t[:])

            # ---- DRAM intermediates ----
            # Full tables are chunk-major: rows [0, 8*R0) hold every core's
            # first AG_MID blocks (chunk A), rows [8*R0, NPAD) the rest.
            # Each chunk's AllGather output is then contiguous, so chunk A
            # can fire mid-phase and overlap the producing phase's tail.
            # The int16 gather-index split rides the same boundary.
            h1_sl = dr.tile([NL, HID], bf16, name="h1_sl")
            h1_f = dr.tile([NPAD, HID], bf16, name="h1_f")
            h2_sl = dr.tile([NL, OUT_F], bf16, name="h2_sl")
            h2_f = dr.tile([NPAD, OUT_F], bf16, name="h2_f")
            s_sl = dr.tile([NL, P], bf16, name="s_sl")
            s_f = dr.tile([NPAD, P], bf16, name="s_f")

            def ag_full(sl, f):
                nc.gpsimd.collective_compute(
                    "AllGather", mybir.AluOpType.bypass,
                    replica_groups=[list(range(NCORES))],
                    ins=[sl.opt()], outs=[f.opt()])

            def tab_half(f, h):
                return f[h * HALF:(h + 1) * HALF, :]

            # ---- gather helper (plain SWDGE gather, Tile-managed sync) ----
            # Emitted with queue_num=0; build_program patches queue_num to
            # (DMASW lane % NQ) after scheduling, so each SWDGE completion
            # semaphore stays bound to exactly one queue while the rings of
            # NQ queues drain in parallel.
            def do_gather_idx(dst_ap, tab_ap, idx_ap, n_idx, **kw):
                nc.gpsimd.dma_gather(
                    dst_ap, tab_ap, idx_ap,
                    n_idx, n_idx, tab_ap.ap[-1][1], **kw)

            def do_gather(dst_ap, tab_ap, idx_cols0, n_idx, **kw):
                do_gather_idx(dst_ap, tab_ap,
                              src16[:, idx_cols0:idx_cols0 + n_idx // 16],
                              n_idx, **kw)

            # ---- Phase A: h1 rows for owned nodes ----
            for b in range(NB):
                xt = sb.tile([P, 2 * P], bf16, tag="xt", name="xt")
                nc.sync.dma_start(xt[:], xT_d[:, b * 2 * P:(b + 1) * 2 * P])
                h1mm = psmm.tile([P, 4 * P], f32, tag="mm", name="h1mm")
                h1ps = h1mm[:, 0:HID]
                for c in range(2):
                    nc.tensor.matmul(
                        out=h1ps, lhsT=xt[:, c * P:(c + 1) * P],
                        rhs=w1_sb[:, c * HID:(c + 1) * HID],
                        start=(c == 0), stop=(c == 1))
                h1sb = sb.tile([P, HID], bf16, tag="h1sb", name="h1sb")
                nc.vector.tensor_copy(h1sb[:], h1ps)
                nc.sync.dma_start(h1_sl[b * P:(b + 1) * P, :], h1sb[:])
            ag_full(h1_sl, h1_f)

            def conv_phase(tab_full3, d_in, per_block_cb):
                """Gather+scatter over all (block, half) groups; one gather
                call per group. per_block_cb(b, aggT_ps_chunks) consumes the
                accumulated block aggregation ([P, P] psum chunks)."""
                nch = d_in // P
                gt0 = 0
                for b in range(NB):
                    T_blk = int(T_arr[b, 0] + T_arr[b, 1])
                    aggT_ps = [ps.tile([P, P], f32, tag=f"aggT{c}",
                                       name=f"aggT{c}") for c in range(nch)]
                    tdone = 0
                    for h in range(2):
                        T = int(T_arr[b, h])
                        tab = tab_half(tab_full3, h)
                        done = 0
                        while done < T:
                            n = min(MAX_GT, T - done)
                            t0 = gt0 + tdone + done
                            g = gp.tile([P, MAX_GT, d_in], bf16, tag="g",
                                        name="g")
                            do_gather(g[:, 0:n, :], tab, 8 * t0, P * n)
                            iw = gp.tile([P, MAX_GT * P], ind_dt, tag="iw",
                                         name="iw")
                            nc.sync.dma_start(
                                iw[:, 0:n * P],
                                ind_d[:, t0 * P:(t0 + n) * P])
                            for tt in range(n):
                                gl = tdone + done + tt
                                for c in range(nch):
                                    nc.tensor.matmul(
                                        out=aggT_ps[c][:],
                                        lhsT=g[:, tt, c * P:(c + 1) * P],
                                        rhs=iw[:, tt * P:(tt + 1) * P],
                                        start=(gl == 0),
                                        stop=(gl == T_blk - 1))
                            done += n
                        tdone += T
                    if T_blk == 0:
                        for c in range(nch):
                            nc.vector.memset(aggT_ps[c][:], 0.0)
                    per_block_cb(b, aggT_ps)
                    gt0 += T_blk

            # ---- Phase B: layer 1 + h2 rows ----
            def l1_block(b, aggT_ps):
                x1T = []
                for c in range(2):
                    z = sb.tile([P, P], f32, tag=f"z{c}", name=f"z{c}")
                    nc.vector.tensor_tensor(
                        out=z[:], in0=aggT_ps[c][:],
                        in1=rsqi_sb[:, b * P:(b + 1) * P],
                        op=mybir.AluOpType.mult)
                    xc = sb.tile([P, P], bf16, tag=f"x1T{c}", name=f"x1T{c}")
                    nc.scalar.activation(xc[:], z[:],
                                         mybir.ActivationFunctionType.Relu,
                                         bias=b1_sb[:, c:c + 1])
                    x1T.append(xc)
                h2mm = psmm.tile([P, 4 * P], f32, tag="mm", name="h2mm")
                h2ps = h2mm[:, 0:OUT_F]
                for c in range(2):
                    nc.tensor.matmul(
                        out=h2ps, lhsT=x1T[c][:],
                        rhs=w2_sb[:, c * OUT_F:(c + 1) * OUT_F],
                        start=(c == 0), stop=(c == 1))
                h2sb = sb.tile([P, OUT_F], bf16, tag="h2sb", name="h2sb")
                nc.vector.tensor_scalar(
                    out=h2sb[:], in0=h2ps, scalar1=rsqo_sb[:, b:b + 1],
                    scalar2=None, op0=mybir.AluOpType.mult)
                nc.sync.dma_start(h2_sl[b * P:(b + 1) * P, :], h2sb[:])

            if PHASE >= 2:
                conv_phase(h1_f, HID, l1_block)
                ag_full(h2_sl, h2_f)

            # ---- Phase C: layer 2 + per-node scores ----
            def l2_block(b, aggT_ps):
                z = sb.tile([P, P], f32, tag="z2", name="z2")
                nc.vector.tensor_tensor(
                    out=z[:], in0=aggT_ps[0][:],
                    in1=rsqi_sb[:, b * P:(b + 1) * P],
                    op=mybir.AluOpType.mult)
                x2T = sb.tile([P, P], f32, tag="x2T", name="x2T")
                nc.scalar.activation(x2T[:], z[:],
                                     mybir.ActivationFunctionType.Relu,
                                     bias=b2_sb[:, 0:1])
                smm = psmm.tile([P, 4 * P], f32, tag="mm", name="smm")
                sps = smm[:, 0:2]
                nc.tensor.matmul(out=sps, lhsT=x2T[:], rhs=wp_sb[:],
                                 start=True, stop=True)
                scp = sb.tile([P, 2], f32, tag="scp", name="scp")
                nc.vector.tensor_copy(scp[:], sps)
                srow = sb.tile([P, P], bf16, tag="srow", name="srow")
                if D_FALLBACK:
                    nc.vector.tensor_scalar(
                        out=srow[:, 0:64], in0=ones_sb[:, 0:64],
                        scalar1=scp[:, 0:1], scalar2=None,
                        op0=mybir.AluOpType.mult)
                    nc.vector.tensor_scalar(
                        out=srow[:, 64:P], in0=ones_sb[:, 0:64],
                        scalar1=scp[:, 1:2], scalar2=None,
                        op0=mybir.AluOpType.mult)
                else:
                    nc.vector.tensor_scalar(
                        out=srow[:], in0=ones_sb[:],
                        scalar1=scp[:, 0:1], scalar2=None,
                        op0=mybir.AluOpType.mult)
                nc.sync.dma_start(s_sl[b * P:(b + 1) * P, :], srow[:])
                if os.environ.get("GCN2_NO_SDCOL", "0") != "1":
                    nc.vector.tensor_copy(sdcol[:, b:b + 1], scp[:, 1:2])

            if PHASE >= 3:
                conv_phase(h2_f, OUT_F, l2_block)
                ag_full(s_sl, s_f)

            # ---- Phase D: per-edge scores ----
            # s_src[src[e]] via one gather of s_f rows ([edge-slot partition,
            # tile, col] layout, col 0 holds the value). s_dst[dst[e]] needs
            # NO gather: per tile, matmul(lhsT=indT tile [dstcol, slot],
            # rhs=sdcol[:, b]) permutes the block's s_dst column into slot
            # order. (In fallback mode s_dst is gathered from s_sl instead.)
            gt0 = 0
            for b in (range(NB) if PHASE >= 4 else []):
                T_blk = int(T_arr[b, 0] + T_arr[b, 1])
                if T_blk == 0:
                    continue
                gA = gp.tile([P, TBMAX, P], bf16, tag="gA", name="gA")
                if D_FALLBACK:
                    gB = gp.tile([P, TBMAX, P], bf16, tag="gB", name="gB")
                else:
                    sd_ps = ps.tile([P, TBMAX], f32, tag="aggT0", name="sd")
                tdone = 0
                for h in range(2):
                    T = int(T_arr[b, h])
                    tab = tab_half(s_f, h)
                    done = 0
                    while done < T:
                        n = min(MAX_GT, T - done)
                        t0 = gt0 + tdone + done
                        sl = tdone + done
                        do_gather(gA[:, sl:sl + n, :], tab, 8 * t0, P * n)
                        if not D_FALLBACK:
                            iw = gp.tile([P, MAX_GT * P], ind_dt, tag="iwT",
                                         name="iwT")
                            nc.sync.dma_start(
                                iw[:, 0:n * P],
                                indT_d[:, t0 * P:(t0 + n) * P])
                            for tt in range(n):
                                nc.tensor.matmul(
                                    out=sd_ps[:, sl + tt:sl + tt + 1],
                                    lhsT=iw[:, tt * P:(tt + 1) * P],
                                    rhs=sdcol[:, b:b + 1],
                                    start=True, stop=True)
                        done += n
                    tdone += T
                if D_FALLBACK:
                    done = 0
                    while done < T_blk:
                        n = min(MAX_GT, T_blk - done)
                        t0 = gt0 + done
                        do_gather_idx(gB[:, done:done + n, :], s_sl[:],
                                      dst16[:, 8 * t0:8 * (t0 + n)], P * n)
                        done += n
                    sd_in = gB[:, 0:T_blk, 64]
                else:
                    sd_in = sd_ps[:, 0:T_blk]
                lsum = sb.tile([P, TBMAX], f32, tag="lsum", name="lsum")
                nc.vector.tensor_tensor(
                    out=lsum[:, 0:T_blk], in0=gA[:, 0:T_blk, 0], in1=sd_in,
                    op=mybir.AluOpType.add)
                scf = sb.tile([P, TBMAX], f32, tag="scf", name="scf")
                nc.scalar.activation(scf[:, 0:T_blk], lsum[:, 0:T_blk],
                                     mybir.ActivationFunctionType.Sigmoid,
                                     bias=bp_sb[:, 0:1])
                nc.sync.dma_start(
                    scoresPT_d[:, gt0:gt0 + T_blk], scf[:, 0:T_blk])
                gt0 += T_blk

    # Post-schedule queue spread: Tile assigned every Pool-engine DMA a
    # DMASW lane (round-robin mod 8) and its completion sem comes from that
    # lane, so queue_num = lane % NQ keeps each sem updated from exactly one
    # queue (the hardware bookkeeping requirement) while NQ queues' rings
    # drain concurrently.
    if NQ > 1:
        import bass_rust as _br
        dmasw0 = _br.PROC_NAMES.index("DMASW0")
        n_patch = 0
        other_lanes = set()
        for f in nc.m.functions:
            for bb in f.blocks:
                for inst in bb.instructions:
                    proc = getattr(inst, "bass_scheduled_proc", None)
                    if proc is None or not (dmasw0 <= proc < dmasw0 + 8):
                        continue
                    if isinstance(inst, mybir.InstDMAGatherAnt):
                        # lane % NQ is the only mapping validated on HW;
                        # lane//2 (pairing lanes onto one queue) hangs.
                        lane = proc - dmasw0
                        if os.environ.get("GCN2_QMAP", "mod") == "pair":
                            inst.queue_num = (lane // 2) % NQ
                        else:
                            inst.queue_num = lane % NQ
                        n_patch += 1
                    else:
                        other_lanes.add(proc - dmasw0)
        # non-gather SWDGE DMAs stay on queue 0; their lanes must map to
        # queue 0 under the same rule or the sem/queue binding breaks.
        assert not other_lanes, (
            f"non-gather SWDGE DMAs on lanes {sorted(other_lanes)} — "
            f"queue mapping must send these lanes to queue 0")
        assert n_patch > 0

    nc.compile()
    return nc


def preprocess(features, src, dst, W1, b1, W2, b2, Wp, bp):
    """Sort/pad edges, build per-core input maps + reassembly info."""
    E = src.shape[0]
    src = src.astype(np.int64)
    dst = dst.astype(np.int64)
    n_nodes = features.shape[0]

    deg_out = np.bincount(src, minlength=n_nodes).astype(np.float64)
    deg_in = np.bincount(dst, minlength=n_nodes).astype(np.float64)
    rsq_out = (1.0 / np.sqrt(np.clip(deg_out, 1.0, None))).astype(np.float32)
    rsq_in = (1.0 / np.sqrt(np.clip(deg_in, 1.0, None))).astype(np.float32)

    NB = BPC
    NL = NB * P
    # edge sort: (dst block, src half, ...)
    gblk = dst // P
    half = (src >= HALF).astype(np.int64)
    key = gblk * 2 + half
    order = np.argsort(key, kind="stable")
    key_s = key[order]
    bounds = np.searchsorted(key_s, np.arange(2 * (NPAD // P) + 1))
    cnt = np.diff(bounds).reshape(NCORES, NB, 2)
    T_arr = np.ceil(cnt.max(axis=0) / P).astype(np.int64)   # [NB, 2]
    TE = int(T_arr.sum())

    src_s = src[order]
    dst_s = dst[order]

    # slot offset of each (b, h) group
    goff = np.zeros((NB, 2), np.int64)
    off = 0
    for b in range(NB):
        for h in range(2):
            goff[b, h] = off
            off += int(T_arr[b, h]) * P
    assert off == TE * P

    slot_src = np.zeros((NCORES, TE * P), np.int64)
    slot_col = np.full((NCORES, TE * P), -1, np.int64)   # -1 = pad
    slot_orig = np.full((NCORES, TE * P), -1, np.int64)

    for core in range(NCORES):
        for b in range(NB):
            g = core * NB + b
            for h in (0, 1):
                lo_e, hi_e = bounds[2 * g + h], bounds[2 * g + h + 1]
                nn = hi_e - lo_e
                o = goff[b, h]
                slot_src[core, o:o + nn] = src_s[lo_e:hi_e]
                slot_col[core, o:o + nn] = dst_s[lo_e:hi_e] - (core * NL + b * P)
                slot_orig[core, o:o + nn] = order[lo_e:hi_e]

    # host-side tensors
    xpad = np.zeros((NPAD, IN_F), np.float32)
    xpad[:n_nodes] = features * rsq_out[:, None]
    rsqi_pad = np.zeros(NPAD, np.float32)
    rsqi_pad[:n_nodes] = rsq_in
    rsqo_pad = np.zeros(NPAD, np.float32)
    rsqo_pad[:n_nodes] = rsq_out

    w1h = np.concatenate([W1[:P, :], W1[P:, :]], axis=1).astype(ml_dtypes.bfloat16)
    w2h = np.concatenate([W2[:P, :], W2[P:, :]], axis=1).astype(ml_dtypes.bfloat16)
    wp12 = np.stack([Wp[:OUT_F, 0], Wp[OUT_F:, 0]], axis=1).astype(np.float32)
    b1c = np.stack([b1[:P], b1[P:]], axis=1).astype(np.float32)
    b2c = b2.astype(np.float32)[:, None]
    bpc = np.full((P, 1), np.float32(bp[0]))

    lane = np.arange(TE * P) % P
    tidx = np.arange(TE * P) // P

    in_maps = []
    for core in range(NCORES):
        base = core * NL
        xT = np.zeros((P, NB * 2 * P), ml_dtypes.bfloat16)
        for b in range(NB):
            blk = xpad[base + b * P: base + (b + 1) * P]   # [128, 256]
            for c in range(2):
                xT[:, (b * 2 + c) * P:(b * 2 + c + 1) * P] = \
                    blk[:, c * P:(c + 1) * P].T.astype(ml_dtypes.bfloat16)
        rsqi_rep = np.broadcast_to(rsqi_pad[base:base + NL][None, :],
                                   (P, NL)).astype(np.float32).copy()
        rsqo_col = rsqo_pad[base:base + NL].reshape(NB, P).T.copy()

        scol = slot_col[core]
        valid = scol >= 0
        hdt = ml_dtypes.bfloat16 if IND_BF16 else ml_dtypes.float8_e4m3
        ind8 = np.zeros((P, TE * P), hdt)
        indT8 = np.zeros((P, TE * P), hdt)
        ind8[lane[valid], tidx[valid] * P + scol[valid]] = 1.0
        indT8[scol[valid], tidx[valid] * P + lane[valid]] = 1.0

        ssrc = slot_src[core]
        s16 = np.where(ssrc >= HALF, ssrc - HALF, ssrc)
        m = dict(
            xT=xT, w1=w1h, w2=w2h, wp12=wp12, b1c=b1c, b2c=b2c, bp=bpc,
            rsqi=rsqi_rep, rsqo=rsqo_col,
            src16=_wrap_idx(s16),
            ind8=ind8, indT8=indT8,
        )
        if D_FALLBACK:
            # local dst node index per slot (block * 128 + col); pads -> 0
            blk_of_slot = np.zeros(TE * P, np.int64)
            off2 = 0
            for b in range(NB):
                w2_ = int(T_arr[b].sum()) * P
                blk_of_slot[off2:off2 + w2_] = b
                off2 += w2_
            dloc = np.where(valid, blk_of_slot * P + np.maximum(scol, 0), 0)
            m["dst16"] = _wrap_idx(dloc)
        in_maps.append(m)

    return in_maps, slot_orig, T_arr, E


_CACHE = {}


def _get_program(T_arr):
    key = (tuple(map(tuple, T_arr)), os.environ.get("GCN2_PHASE", "4"),
           os.environ.get("GCN2_MAXGT"), os.environ.get("GCN2_AG_MID"),
           IND_BF16, D_FALLBACK, NQ, SCRATCH)
    if key not in _CACHE:
        _CACHE[key] = build_program(np.asarray(T_arr))
    return _CACHE[key]


def kernel(features, src, dst, edge_type, W1, b1, W2, b2, Wp, bp, _trace=False,
           _tmpdir=None):
    features = np.asarray(features, np.float32)
    src_i = np.asarray(src, np.int32)
    dst_i = np.asarray(dst, np.int32)
    in_maps, slot_orig, T_arr, E = preprocess(
        features, src_i, dst_i, np.asarray(W1), np.asarray(b1),
        np.asarray(W2), np.asarray(b2), np.asarray(Wp), np.asarray(bp))
    nc = _get_program(T_arr)
    res = run_bass_kernel_spmd(nc, in_maps, core_ids=list(range(NCORES)),
                               trace=_trace, tmpdir=_tmpdir)
    out = np.zeros(E, np.float32)
    for core in range(NCORES):
        sc = np.asarray(res.results[core]["scoresPT"])   # [P, TE]
        flat = sc.T.reshape(-1)
        so = slot_orig[core]
        m = so >= 0
        out[so[m]] = flat[m[:flat.shape[0]] if m.shape[0] > flat.shape[0] else m]
    if _trace:
        kernel._last_results = res
    return out


# revision 50
# speedup vs baseline: 1.2658x; 1.2658x over previous
"""GCN (2-layer GraphConv + edge scorer) on 8 Trainium2 NeuronCores — v4.

The bottleneck is Pool-engine SWDGE descriptor generation for the per-edge
row gathers (~4.5-8.4 ns/row, strictly serial on the engine), so the design
minimizes gather passes (3: layer-1 rows, layer-2 rows, score rows) and
keeps every other engine hidden underneath them.

  - Nodes padded to 50176 = 8 x 49 x 128; core i owns dst nodes
    [i*6272, (i+1)*6272). Edges sorted (dst block, src half); per-(block,
    half) groups padded to whole 128-edge tiles (counts = max over cores so
    one SPMD program fits all).
  - Gathers are issued in <=5-tile (640-index) calls so each call's
    descriptors fit one SWDGE ring outright; after Tile scheduling,
    build_program patches each gather's queue_num to its assigned DMASW
    lane % NQ, spreading calls over the 4 SWDGE queues (sem/queue binding
    stays sound) so ring drains proceed in parallel with desc-gen.
  - Normalization split: rsq_out folded into X on the host, rsq_in applied
    post-aggregation per dst block. One-hot scatter tiles are pure 0/1,
    host-precomputed in bf16 (TensorE fast-path) and streamed.
  - h1 = (X*rsq_out)@W1 computed per owned block, AllGathered in bf16;
    TensorE scatter-accumulates aggT per block in PSUM via one-hot matmul.
  - Layer 2 gathers h2 = (x1*rsq_out)@W2 rows (128-wide bf16), so gather
    traffic and scatter matmuls are halved vs gathering x1.
  - Scores: s_src/s_dst per node from x2T via one matmul; s_src replicated
    into a [node,128]-bf16 table (tensor_scalar x ones), AllGathered, and
    fetched per edge with a normal gather (value in column 0 of each row);
    s_dst needs no gather — per tile, matmul(lhsT=indT one-hot, rhs=sdcol)
    permutes the block's s_dst column into edge-slot order. sigmoid(+bp)
    on ScalarE, output in [partition=slot%128, tile] layout.
Host does index-only preprocessing and reassembles per-core score slots
into the original edge order.
"""
import os
import sys

_REPO = os.environ.get("TRN_RL_REPO", "/opt/trn_rl_repo")
if _REPO not in sys.path:
    sys.path.insert(0, _REPO)

import ml_dtypes
import numpy as np

import concourse.bacc as bacc
import concourse.tile as tile
from concourse import mybir
from concourse.bass_utils import run_bass_kernel_spmd

P = 128
NCORES = 8
N_NODES = 50000
NPAD = 50176            # 8 * 49 * 128
BPC = NPAD // NCORES // P   # blocks per core = 49
HALF = NPAD // 2        # int16 gather index limit -> lo/hi table split
IN_F = 256
HID = 256
OUT_F = 128

f32 = mybir.dt.float32
bf16 = mybir.dt.bfloat16
i16 = mybir.dt.int16
fp8 = mybir.dt.float8e4
IND_BF16 = os.environ.get("GCN2_IND_BF16", "1") == "1"
ind_dt = bf16 if IND_BF16 else fp8
D_FALLBACK = os.environ.get("GCN2_D_FALLBACK", "0") == "1"
NQ = int(os.environ.get("GCN2_NQ", "4"))       # SWDGE queues to cycle
SCRATCH = int(os.environ.get("GCN2_SCRATCH", "16384"))


def _wrap_idx(idx_flat):
    """dma_gather index layout: idx k -> [k%16, k//16], replicated 8x."""
    n = idx_flat.shape[0]
    w = idx_flat.reshape(n // 16, 16).T
    return np.tile(w, (8, 1)).astype(np.int16)


def build_program(T_arr, trace_label="gcn2"):
    """One SPMD program for all cores. T_arr: [BPC, 2] per-(block, half)
    tile counts (shared across cores)."""
    NB = T_arr.shape[0]
    TE = int(T_arr.sum())                 # total edge tiles per core
    NL = NB * P                           # nodes per core
    # tiles per gather call. Hard cap: 1024 indices (8 tiles) — the SWDGE
    # descriptor ring holds ~65 descs/engine and a single bigger call
    # deadlocks in await_space. Default 5 so a 9-tile group splits 5+4 and
    # BOTH calls fit the ring outright: descriptor-gen then never throttles
    # to the ring-drain rate mid-call (measured 2.33ms -> 2.07ms).
    MAX_GT = int(os.environ.get("GCN2_MAXGT", str(min(5, int(T_arr.max())))))
    PHASE = int(os.environ.get("GCN2_PHASE", "4"))
    AG_MID = int(os.environ.get("GCN2_AG_MID", "25"))  # blocks in chunk A
    AG_OVL = os.environ.get("GCN2_AG_OVL", "1") == "1"  # fire chunk A mid-phase

    nc = bacc.Bacc("TRN2", target_bir_lowering=False, debug=False,
                   enable_asserts=True, num_devices=NCORES,
                   num_swdge_queues=NQ,
                   dynamic_dma_scratch_size=SCRATCH)

    xT_d = nc.dram_tensor("xT", [P, NB * 2 * P], bf16, kind="ExternalInput")
    w1_d = nc.dram_tensor("w1", [P, 2 * HID], bf16, kind="ExternalInput")
    w2_d = nc.dram_tensor("w2", [P, 2 * OUT_F], bf16, kind="ExternalInput")
    wp_d = nc.dram_tensor("wp12", [P, 2], f32, kind="ExternalInput")
    b1_d = nc.dram_tensor("b1c", [P, 2], f32, kind="ExternalInput")
    b2_d = nc.dram_tensor("b2c", [P, 1], f32, kind="ExternalInput")
    bp_d = nc.dram_tensor("bp", [P, 1], f32, kind="ExternalInput")
    rsqi_d = nc.dram_tensor("rsqi", [P, NL], f32, kind="ExternalInput")
    rsqo_d = nc.dram_tensor("rsqo", [P, NB], f32, kind="ExternalInput")
    src16_d = nc.dram_tensor("src16", [P, 8 * TE], i16, kind="ExternalInput")
    ind_d = nc.dram_tensor("ind8", [P, TE * P], ind_dt, kind="ExternalInput")
    indT_d = nc.dram_tensor("indT8", [P, TE * P], ind_dt, kind="ExternalInput")
    if D_FALLBACK:
        dst16_d = nc.dram_tensor("dst16", [P, 8 * TE], i16,
                                 kind="ExternalInput")
    scoresPT_d = nc.dram_tensor("scoresPT", [P, TE], f32,
                                kind="ExternalOutput")
    TBMAX = int((T_arr[:, 0] + T_arr[:, 1]).max())
    # AllGather outputs in the pair-HBM shared address space: the collective
    # takes its fast HBM-HBM path when the output tensor is Shared.
    SHARED = os.environ.get("GCN2_SHARED", "1") == "1"
    if SHARED:
        h1_fS = nc.dram_tensor("h1_fS", [NPAD, HID], bf16,
                               addr_space="Shared")
        h2_fS = nc.dram_tensor("h2_fS", [NPAD, OUT_F], bf16,
                               addr_space="Shared")
        s_fS = nc.dram_tensor("s_fS", [NPAD, P], bf16, addr_space="Shared")

    with tile.TileContext(nc) as tc:
        with (
            tc.tile_pool(name="cons", bufs=1) as cons,
            tc.tile_pool(name="sb", bufs=3) as sb,
            tc.tile_pool(name="gp",
                         bufs=int(os.environ.get("GCN2_GPBUFS", "8"))) as gp,
            tc.tile_pool(name="ps", bufs=3, space="PSUM") as ps,
            tc.tile_pool(name="psmm", bufs=2, space="PSUM") as psmm,
            tc.tile_pool(name="dram", bufs=1, space="DRAM") as dr,
        ):
            # ---- resident constants ----
            w1_sb = cons.tile([P, 2 * HID], bf16, name="w1_sb")
            w2_sb = cons.tile([P, 2 * OUT_F], bf16, name="w2_sb")
            wp_sb = cons.tile([P, 2], f32, name="wp_sb")
            b1_sb = cons.tile([P, 2], f32, name="b1_sb")
            b2_sb = cons.tile([P, 1], f32, name="b2_sb")
            bp_sb = cons.tile([P, 1], f32, name="bp_sb")
            rsqi_sb = cons.tile([P, NL], f32, name="rsqi_sb")
            rsqo_sb = cons.tile([P, NB], f32, name="rsqo_sb")
            src16 = cons.tile([P, 8 * TE], i16, name="src16")
            if D_FALLBACK:
                dst16 = cons.tile([P, 8 * TE], i16, name="dst16")
                nc.sync.dma_start(dst16[:], dst16_d[:])
            sdcol = cons.tile([P, NB], bf16, name="sdcol")
            ones_sb = cons.tile([P, P], bf16, name="ones_sb")
            nc.vector.memset(ones_sb[:], 1.0)
            for s_t, d_t in [(w1_sb, w1_d), (w2_sb, w2_d), (wp_sb, wp_d),
                             (b1_sb, b1_d), (b2_sb, b2_d), (bp_sb, bp_d),
                             (rsqi_sb, rsqi_d), (rsqo_sb, rsqo_d),
                             (src16, src16_d)]:
                nc.sync.dma_start(s_t[:], d_t[:])

            # ---- DRAM intermediates ----
            # Full tables are chunk-major: rows [0, 8*R0) hold every core's
            # first AG_MID blocks (chunk A), rows [8*R0, NPAD) the rest.
            # Each chunk's AllGather output is then contiguous, so chunk A
            # can fire mid-phase and overlap the producing phase's tail.
            # The int16 gather-index split rides the same boundary.
            h1_sl = dr.tile([NL, HID], bf16, name="h1_sl")
            h2_sl = dr.tile([NL, OUT_F], bf16, name="h2_sl")
            s_sl = dr.tile([NL, P], bf16, name="s_sl")
            if SHARED:
                h1_f, h2_f, s_f = h1_fS, h2_fS, s_fS
            else:
                h1_f = dr.tile([NPAD, HID], bf16, name="h1_f")
                h2_f = dr.tile([NPAD, OUT_F], bf16, name="h2_f")
                s_f = dr.tile([NPAD, P], bf16, name="s_f")

            def ag_full(sl, f):
                nc.gpsimd.collective_compute(
                    "AllGather", mybir.AluOpType.bypass,
                    replica_groups=[list(range(NCORES))],
                    ins=[sl.opt()], outs=[f[0:NPAD, :]])

            def tab_half(f, h):
                return f[h * HALF:(h + 1) * HALF, :]

            # ---- gather helper (plain SWDGE gather, Tile-managed sync) ----
            # Emitted with queue_num=0; build_program patches queue_num to
            # (DMASW lane % NQ) after scheduling, so each SWDGE completion
            # semaphore stays bound to exactly one queue while the rings of
            # NQ queues drain in parallel.
            def do_gather_idx(dst_ap, tab_ap, idx_ap, n_idx, **kw):
                nc.gpsimd.dma_gather(
                    dst_ap, tab_ap, idx_ap,
                    n_idx, n_idx, tab_ap.ap[-1][1], **kw)

            def do_gather(dst_ap, tab_ap, idx_cols0, n_idx, **kw):
                do_gather_idx(dst_ap, tab_ap,
                              src16[:, idx_cols0:idx_cols0 + n_idx // 16],
                              n_idx, **kw)

            # ---- Phase A: h1 rows for owned nodes ----
            for b in range(NB):
                xt = sb.tile([P, 2 * P], bf16, tag="xt", name="xt")
                nc.sync.dma_start(xt[:], xT_d[:, b * 2 * P:(b + 1) * 2 * P])
                h1mm = psmm.tile([P, 4 * P], f32, tag="mm", name="h1mm")
                h1ps = h1mm[:, 0:HID]
                for c in range(2):
                    nc.tensor.matmul(
                        out=h1ps, lhsT=xt[:, c * P:(c + 1) * P],
                        rhs=w1_sb[:, c * HID:(c + 1) * HID],
                        start=(c == 0), stop=(c == 1))
                h1sb = sb.tile([P, HID], bf16, tag="h1sb", name="h1sb")
                nc.vector.tensor_copy(h1sb[:], h1ps)
                nc.sync.dma_start(h1_sl[b * P:(b + 1) * P, :], h1sb[:])
            ag_full(h1_sl, h1_f)

            def conv_phase(tab_full3, d_in, per_block_cb):
                """Gather+scatter over all (block, half) groups; one gather
                call per group. per_block_cb(b, aggT_ps_chunks) consumes the
                accumulated block aggregation ([P, P] psum chunks)."""
                nch = d_in // P
                gt0 = 0
                for b in range(NB):
                    T_blk = int(T_arr[b, 0] + T_arr[b, 1])
                    aggT_ps = [ps.tile([P, P], f32, tag=f"aggT{c}",
                                       name=f"aggT{c}") for c in range(nch)]
                    tdone = 0
                    for h in range(2):
                        T = int(T_arr[b, h])
                        tab = tab_half(tab_full3, h)
                        done = 0
                        while done < T:
                            n = min(MAX_GT, T - done)
                            t0 = gt0 + tdone + done
                            g = gp.tile([P, MAX_GT, d_in], bf16, tag="g",
                                        name="g")
                            do_gather(g[:, 0:n, :], tab, 8 * t0, P * n)
                            iw = gp.tile([P, MAX_GT * P], ind_dt, tag="iw",
                                         name="iw")
                            nc.sync.dma_start(
                                iw[:, 0:n * P],
                                ind_d[:, t0 * P:(t0 + n) * P])
                            for tt in range(n):
                                gl = tdone + done + tt
                                for c in range(nch):
                                    nc.tensor.matmul(
                                        out=aggT_ps[c][:],
                                        lhsT=g[:, tt, c * P:(c + 1) * P],
                                        rhs=iw[:, tt * P:(tt + 1) * P],
                                        start=(gl == 0),
                                        stop=(gl == T_blk - 1))
                            done += n
                        tdone += T
                    if T_blk == 0:
                        for c in range(nch):
                            nc.vector.memset(aggT_ps[c][:], 0.0)
                    per_block_cb(b, aggT_ps)
                    gt0 += T_blk

            # ---- Phase B: layer 1 + h2 rows ----
            def l1_block(b, aggT_ps):
                x1T = []
                for c in range(2):
                    z = sb.tile([P, P], f32, tag=f"z{c}", name=f"z{c}")
                    nc.vector.tensor_tensor(
                        out=z[:], in0=aggT_ps[c][:],
                        in1=rsqi_sb[:, b * P:(b + 1) * P],
                        op=mybir.AluOpType.mult)
                    xc = sb.tile([P, P], bf16, tag=f"x1T{c}", name=f"x1T{c}")
                    nc.scalar.activation(xc[:], z[:],
                                         mybir.ActivationFunctionType.Relu,
                                         bias=b1_sb[:, c:c + 1])
                    x1T.append(xc)
                h2mm = psmm.tile([P, 4 * P], f32, tag="mm", name="h2mm")
                h2ps = h2mm[:, 0:OUT_F]
                for c in range(2):
                    nc.tensor.matmul(
                        out=h2ps, lhsT=x1T[c][:],
                        rhs=w2_sb[:, c * OUT_F:(c + 1) * OUT_F],
                        start=(c == 0), stop=(c == 1))
                h2sb = sb.tile([P, OUT_F], bf16, tag="h2sb", name="h2sb")
                nc.vector.tensor_scalar(
                    out=h2sb[:], in0=h2ps, scalar1=rsqo_sb[:, b:b + 1],
                    scalar2=None, op0=mybir.AluOpType.mult)
                nc.sync.dma_start(h2_sl[b * P:(b + 1) * P, :], h2sb[:])

            if PHASE >= 2:
                conv_phase(h1_f, HID, l1_block)
                ag_full(h2_sl, h2_f)

            # ---- Phase C: layer 2 + per-node scores ----
            def l2_block(b, aggT_ps):
                z = sb.tile([P, P], f32, tag="z2", name="z2")
                nc.vector.tensor_tensor(
                    out=z[:], in0=aggT_ps[0][:],
                    in1=rsqi_sb[:, b * P:(b + 1) * P],
                    op=mybir.AluOpType.mult)
                x2T = sb.tile([P, P], f32, tag="x2T", name="x2T")
                nc.scalar.activation(x2T[:], z[:],
                                     mybir.ActivationFunctionType.Relu,
                                     bias=b2_sb[:, 0:1])
                smm = psmm.tile([P, 4 * P], f32, tag="mm", name="smm")
                sps = smm[:, 0:2]
                nc.tensor.matmul(out=sps, lhsT=x2T[:], rhs=wp_sb[:],
                                 start=True, stop=True)
                scp = sb.tile([P, 2], f32, tag="scp", name="scp")
                nc.vector.tensor_copy(scp[:], sps)
                srow = sb.tile([P, P], bf16, tag="srow", name="srow")
                if D_FALLBACK:
                    nc.vector.tensor_scalar(
                        out=srow[:, 0:64], in0=ones_sb[:, 0:64],
                        scalar1=scp[:, 0:1], scalar2=None,
                        op0=mybir.AluOpType.mult)
                    nc.vector.tensor_scalar(
                        out=srow[:, 64:P], in0=ones_sb[:, 0:64],
                        scalar1=scp[:, 1:2], scalar2=None,
                        op0=mybir.AluOpType.mult)
                else:
                    nc.vector.tensor_scalar(
                        out=srow[:], in0=ones_sb[:],
                        scalar1=scp[:, 0:1], scalar2=None,
                        op0=mybir.AluOpType.mult)
                nc.sync.dma_start(s_sl[b * P:(b + 1) * P, :], srow[:])
                if os.environ.get("GCN2_NO_SDCOL", "0") != "1":
                    nc.vector.tensor_copy(sdcol[:, b:b + 1], scp[:, 1:2])

            if PHASE >= 3:
                conv_phase(h2_f, OUT_F, l2_block)
                ag_full(s_sl, s_f)

            # ---- Phase D: per-edge scores ----
            # s_src[src[e]] via one gather of s_f rows ([edge-slot partition,
            # tile, col] layout, col 0 holds the value). s_dst[dst[e]] needs
            # NO gather: per tile, matmul(lhsT=indT tile [dstcol, slot],
            # rhs=sdcol[:, b]) permutes the block's s_dst column into slot
            # order. (In fallback mode s_dst is gathered from s_sl instead.)
            gt0 = 0
            for b in (range(NB) if PHASE >= 4 else []):
                T_blk = int(T_arr[b, 0] + T_arr[b, 1])
                if T_blk == 0:
                    continue
                gA = gp.tile([P, TBMAX, P], bf16, tag="gA", name="gA")
                if D_FALLBACK:
                    gB = gp.tile([P, TBMAX, P], bf16, tag="gB", name="gB")
                else:
                    sd_ps = ps.tile([P, TBMAX], f32, tag="aggT0", name="sd")
                tdone = 0
                for h in range(2):
                    T = int(T_arr[b, h])
                    tab = tab_half(s_f, h)
                    done = 0
                    while done < T:
                        n = min(MAX_GT, T - done)
                        t0 = gt0 + tdone + done
                        sl = tdone + done
                        do_gather(gA[:, sl:sl + n, :], tab, 8 * t0, P * n)
                        if not D_FALLBACK:
                            iw = gp.tile([P, MAX_GT * P], ind_dt, tag="iwT",
                                         name="iwT")
                            nc.sync.dma_start(
                                iw[:, 0:n * P],
                                indT_d[:, t0 * P:(t0 + n) * P])
                            for tt in range(n):
                                nc.tensor.matmul(
                                    out=sd_ps[:, sl + tt:sl + tt + 1],
                                    lhsT=iw[:, tt * P:(tt + 1) * P],
                                    rhs=sdcol[:, b:b + 1],
                                    start=True, stop=True)
                        done += n
                    tdone += T
                if D_FALLBACK:
                    done = 0
                    while done < T_blk:
                        n = min(MAX_GT, T_blk - done)
                        t0 = gt0 + done
                        do_gather_idx(gB[:, done:done + n, :], s_sl[:],
                                      dst16[:, 8 * t0:8 * (t0 + n)], P * n)
                        done += n
                    sd_in = gB[:, 0:T_blk, 64]
                else:
                    sd_in = sd_ps[:, 0:T_blk]
                lsum = sb.tile([P, TBMAX], f32, tag="lsum", name="lsum")
                nc.vector.tensor_tensor(
                    out=lsum[:, 0:T_blk], in0=gA[:, 0:T_blk, 0], in1=sd_in,
                    op=mybir.AluOpType.add)
                scf = sb.tile([P, TBMAX], f32, tag="scf", name="scf")
                nc.scalar.activation(scf[:, 0:T_blk], lsum[:, 0:T_blk],
                                     mybir.ActivationFunctionType.Sigmoid,
                                     bias=bp_sb[:, 0:1])
                nc.sync.dma_start(
                    scoresPT_d[:, gt0:gt0 + T_blk], scf[:, 0:T_blk])
                gt0 += T_blk

    # Post-schedule queue spread: Tile assigned every Pool-engine DMA a
    # DMASW lane (round-robin mod 8) and its completion sem comes from that
    # lane, so queue_num = lane % NQ keeps each sem updated from exactly one
    # queue (the hardware bookkeeping requirement) while NQ queues' rings
    # drain concurrently.
    if NQ > 1:
        import bass_rust as _br
        dmasw0 = _br.PROC_NAMES.index("DMASW0")
        n_patch = 0
        other_lanes = set()
        for f in nc.m.functions:
            for bb in f.blocks:
                for inst in bb.instructions:
                    proc = getattr(inst, "bass_scheduled_proc", None)
                    if proc is None or not (dmasw0 <= proc < dmasw0 + 8):
                        continue
                    if isinstance(inst, mybir.InstDMAGatherAnt):
                        # lane % NQ is the only mapping validated on HW;
                        # lane//2 (pairing lanes onto one queue) hangs.
                        lane = proc - dmasw0
                        if os.environ.get("GCN2_QMAP", "mod") == "pair":
                            inst.queue_num = (lane // 2) % NQ
                        else:
                            inst.queue_num = lane % NQ
                        n_patch += 1
                    else:
                        other_lanes.add(proc - dmasw0)
        # non-gather SWDGE DMAs stay on queue 0; their lanes must map to
        # queue 0 under the same rule or the sem/queue binding breaks.
        assert not other_lanes, (
            f"non-gather SWDGE DMAs on lanes {sorted(other_lanes)} — "
            f"queue mapping must send these lanes to queue 0")
        assert n_patch > 0

    nc.compile()
    return nc


def preprocess(features, src, dst, W1, b1, W2, b2, Wp, bp):
    """Sort/pad edges, build per-core input maps + reassembly info."""
    E = src.shape[0]
    src = src.astype(np.int64)
    dst = dst.astype(np.int64)
    n_nodes = features.shape[0]

    deg_out = np.bincount(src, minlength=n_nodes).astype(np.float64)
    deg_in = np.bincount(dst, minlength=n_nodes).astype(np.float64)
    rsq_out = (1.0 / np.sqrt(np.clip(deg_out, 1.0, None))).astype(np.float32)
    rsq_in = (1.0 / np.sqrt(np.clip(deg_in, 1.0, None))).astype(np.float32)

    NB = BPC
    NL = NB * P
    # edge sort: (dst block, src half, ...)
    gblk = dst // P
    half = (src >= HALF).astype(np.int64)
    key = gblk * 2 + half
    order = np.argsort(key, kind="stable")
    key_s = key[order]
    bounds = np.searchsorted(key_s, np.arange(2 * (NPAD // P) + 1))
    cnt = np.diff(bounds).reshape(NCORES, NB, 2)
    T_arr = np.ceil(cnt.max(axis=0) / P).astype(np.int64)   # [NB, 2]
    TE = int(T_arr.sum())

    src_s = src[order]
    dst_s = dst[order]

    # slot offset of each (b, h) group
    goff = np.zeros((NB, 2), np.int64)
    off = 0
    for b in range(NB):
        for h in range(2):
            goff[b, h] = off
            off += int(T_arr[b, h]) * P
    assert off == TE * P

    slot_src = np.zeros((NCORES, TE * P), np.int64)
    slot_col = np.full((NCORES, TE * P), -1, np.int64)   # -1 = pad
    slot_orig = np.full((NCORES, TE * P), -1, np.int64)

    for core in range(NCORES):
        for b in range(NB):
            g = core * NB + b
            for h in (0, 1):
                lo_e, hi_e = bounds[2 * g + h], bounds[2 * g + h + 1]
                nn = hi_e - lo_e
                o = goff[b, h]
                slot_src[core, o:o + nn] = src_s[lo_e:hi_e]
                slot_col[core, o:o + nn] = dst_s[lo_e:hi_e] - (core * NL + b * P)
                slot_orig[core, o:o + nn] = order[lo_e:hi_e]

    # host-side tensors
    xpad = np.zeros((NPAD, IN_F), np.float32)
    xpad[:n_nodes] = features * rsq_out[:, None]
    rsqi_pad = np.zeros(NPAD, np.float32)
    rsqi_pad[:n_nodes] = rsq_in
    rsqo_pad = np.zeros(NPAD, np.float32)
    rsqo_pad[:n_nodes] = rsq_out

    w1h = np.concatenate([W1[:P, :], W1[P:, :]], axis=1).astype(ml_dtypes.bfloat16)
    w2h = np.concatenate([W2[:P, :], W2[P:, :]], axis=1).astype(ml_dtypes.bfloat16)
    wp12 = np.stack([Wp[:OUT_F, 0], Wp[OUT_F:, 0]], axis=1).astype(np.float32)
    b1c = np.stack([b1[:P], b1[P:]], axis=1).astype(np.float32)
    b2c = b2.astype(np.float32)[:, None]
    bpc = np.full((P, 1), np.float32(bp[0]))

    lane = np.arange(TE * P) % P
    tidx = np.arange(TE * P) // P

    in_maps = []
    for core in range(NCORES):
        base = core * NL
        xT = np.zeros((P, NB * 2 * P), ml_dtypes.bfloat16)
        for b in range(NB):
            blk = xpad[base + b * P: base + (b + 1) * P]   # [128, 256]
            for c in range(2):
                xT[:, (b * 2 + c) * P:(b * 2 + c + 1) * P] = \
                    blk[:, c * P:(c + 1) * P].T.astype(ml_dtypes.bfloat16)
        rsqi_rep = np.broadcast_to(rsqi_pad[base:base + NL][None, :],
                                   (P, NL)).astype(np.float32).copy()
        rsqo_col = rsqo_pad[base:base + NL].reshape(NB, P).T.copy()

        scol = slot_col[core]
        valid = scol >= 0
        hdt = ml_dtypes.bfloat16 if IND_BF16 else ml_dtypes.float8_e4m3
        ind8 = np.zeros((P, TE * P), hdt)
        indT8 = np.zeros((P, TE * P), hdt)
        ind8[lane[valid], tidx[valid] * P + scol[valid]] = 1.0
        indT8[scol[valid], tidx[valid] * P + lane[valid]] = 1.0

        ssrc = slot_src[core]
        s16 = np.where(ssrc >= HALF, ssrc - HALF, ssrc)
        m = dict(
            xT=xT, w1=w1h, w2=w2h, wp12=wp12, b1c=b1c, b2c=b2c, bp=bpc,
            rsqi=rsqi_rep, rsqo=rsqo_col,
            src16=_wrap_idx(s16),
            ind8=ind8, indT8=indT8,
        )
        if D_FALLBACK:
            # local dst node index per slot (block * 128 + col); pads -> 0
            blk_of_slot = np.zeros(TE * P, np.int64)
            off2 = 0
            for b in range(NB):
                w2_ = int(T_arr[b].sum()) * P
                blk_of_slot[off2:off2 + w2_] = b
                off2 += w2_
            dloc = np.where(valid, blk_of_slot * P + np.maximum(scol, 0), 0)
            m["dst16"] = _wrap_idx(dloc)
        in_maps.append(m)

    return in_maps, slot_orig, T_arr, E


_CACHE = {}


def _get_program(T_arr):
    key = (tuple(map(tuple, T_arr)), os.environ.get("GCN2_PHASE", "4"),
           os.environ.get("GCN2_MAXGT"), os.environ.get("GCN2_AG_MID"),
           os.environ.get("GCN2_SHARED"), os.environ.get("GCN2_GPBUFS"),
           IND_BF16, D_FALLBACK, NQ, SCRATCH)
    if key not in _CACHE:
        _CACHE[key] = build_program(np.asarray(T_arr))
    return _CACHE[key]


def kernel(features, src, dst, edge_type, W1, b1, W2, b2, Wp, bp, _trace=False,
           _tmpdir=None):
    features = np.asarray(features, np.float32)
    src_i = np.asarray(src, np.int32)
    dst_i = np.asarray(dst, np.int32)
    in_maps, slot_orig, T_arr, E = preprocess(
        features, src_i, dst_i, np.asarray(W1), np.asarray(b1),
        np.asarray(W2), np.asarray(b2), np.asarray(Wp), np.asarray(bp))
    nc = _get_program(T_arr)
    res = run_bass_kernel_spmd(nc, in_maps, core_ids=list(range(NCORES)),
                               trace=_trace, tmpdir=_tmpdir)
    out = np.zeros(E, np.float32)
    for core in range(NCORES):
        sc = np.asarray(res.results[core]["scoresPT"])   # [P, TE]
        flat = sc.T.reshape(-1)
        so = slot_orig[core]
        m = so >= 0
        out[so[m]] = flat[m[:flat.shape[0]] if m.shape[0] > flat.shape[0] else m]
    if _trace:
        kernel._last_results = res
    return out


# revision 52
# speedup vs baseline: 1.3043x; 1.0304x over previous
"""GCN (2-layer GraphConv + edge scorer) on 8 Trainium2 NeuronCores — v4.

The bottleneck is Pool-engine SWDGE descriptor generation for the per-edge
row gathers (~4.5-8.4 ns/row, strictly serial on the engine), so the design
minimizes gather passes (3: layer-1 rows, layer-2 rows, score rows) and
keeps every other engine hidden underneath them.

  - Nodes padded to 50176 = 8 x 49 x 128; core i owns dst nodes
    [i*6272, (i+1)*6272). Edges sorted (dst block, src half); per-(block,
    half) groups padded to whole 128-edge tiles (counts = max over cores so
    one SPMD program fits all).
  - Gathers are issued in <=5-tile (640-index) calls so each call's
    descriptors fit one SWDGE ring outright; after Tile scheduling,
    build_program patches each gather's queue_num to its assigned DMASW
    lane % NQ, spreading calls over the 4 SWDGE queues (sem/queue binding
    stays sound) so ring drains proceed in parallel with desc-gen.
  - Normalization split: rsq_out folded into X on the host, rsq_in applied
    post-aggregation per dst block. One-hot scatter tiles are pure 0/1,
    host-precomputed in bf16 (TensorE fast-path) and streamed.
  - h1 = (X*rsq_out)@W1 computed per owned block, AllGathered in bf16;
    TensorE scatter-accumulates aggT per block in PSUM via one-hot matmul.
  - Layer 2 gathers h2 = (x1*rsq_out)@W2 rows (128-wide bf16), so gather
    traffic and scatter matmuls are halved vs gathering x1.
  - Scores: s_src/s_dst per node from x2T via one matmul; s_src replicated
    into a [node,128]-bf16 table (tensor_scalar x ones), AllGathered, and
    fetched per edge with a normal gather (value in column 0 of each row);
    s_dst needs no gather — per tile, matmul(lhsT=indT one-hot, rhs=sdcol)
    permutes the block's s_dst column into edge-slot order. sigmoid(+bp)
    on ScalarE, output in [partition=slot%128, tile] layout.
Host does index-only preprocessing and reassembles per-core score slots
into the original edge order.
"""
import os
import sys

_REPO = os.environ.get("TRN_RL_REPO", "/opt/trn_rl_repo")
if _REPO not in sys.path:
    sys.path.insert(0, _REPO)

import ml_dtypes
import numpy as np

import concourse.bacc as bacc
import concourse.tile as tile
from concourse import mybir
from concourse.bass_utils import run_bass_kernel_spmd

P = 128
NCORES = 8
N_NODES = 50000
NPAD = 50176            # 8 * 49 * 128
BPC = NPAD // NCORES // P   # blocks per core = 49
HALF = NPAD // 2        # int16 gather index limit -> lo/hi table split
IN_F = 256
HID = 256
OUT_F = 128

f32 = mybir.dt.float32
bf16 = mybir.dt.bfloat16
i16 = mybir.dt.int16
fp8 = mybir.dt.float8e4
IND_BF16 = os.environ.get("GCN2_IND_BF16", "1") == "1"
ind_dt = bf16 if IND_BF16 else fp8
D_FALLBACK = os.environ.get("GCN2_D_FALLBACK", "0") == "1"
NQ = int(os.environ.get("GCN2_NQ", "4"))       # SWDGE queues to cycle
SCRATCH = int(os.environ.get("GCN2_SCRATCH", "16384"))


def _wrap_idx(idx_flat):
    """dma_gather index layout: idx k -> [k%16, k//16], replicated 8x."""
    n = idx_flat.shape[0]
    w = idx_flat.reshape(n // 16, 16).T
    return np.tile(w, (8, 1)).astype(np.int16)


def build_program(T_arr, trace_label="gcn2"):
    """One SPMD program for all cores. T_arr: [BPC, 2] per-(block, half)
    tile counts (shared across cores)."""
    NB = T_arr.shape[0]
    TE = int(T_arr.sum())                 # total edge tiles per core
    NL = NB * P                           # nodes per core
    # tiles per gather call. Hard cap: 1024 indices (8 tiles) — the SWDGE
    # descriptor ring holds ~65 descs/engine and a single bigger call
    # deadlocks in await_space. Default 5 so a 9-tile group splits 5+4 and
    # BOTH calls fit the ring outright: descriptor-gen then never throttles
    # to the ring-drain rate mid-call (measured 2.33ms -> 2.07ms).
    MAX_GT = int(os.environ.get("GCN2_MAXGT", str(min(5, int(T_arr.max())))))
    PHASE = int(os.environ.get("GCN2_PHASE", "4"))
    AG_MID = int(os.environ.get("GCN2_AG_MID", "25"))  # blocks in chunk A
    AG_OVL = os.environ.get("GCN2_AG_OVL", "1") == "1"  # fire chunk A mid-phase

    nc = bacc.Bacc("TRN2", target_bir_lowering=False, debug=False,
                   enable_asserts=True, num_devices=NCORES,
                   num_swdge_queues=NQ,
                   dynamic_dma_scratch_size=SCRATCH)

    xT_d = nc.dram_tensor("xT", [P, NB * 2 * P], bf16, kind="ExternalInput")
    w1_d = nc.dram_tensor("w1", [P, 2 * HID], bf16, kind="ExternalInput")
    w2_d = nc.dram_tensor("w2", [P, 2 * OUT_F], bf16, kind="ExternalInput")
    wp_d = nc.dram_tensor("wp12", [P, 2], f32, kind="ExternalInput")
    b1_d = nc.dram_tensor("b1c", [P, 2], f32, kind="ExternalInput")
    b2_d = nc.dram_tensor("b2c", [P, 1], f32, kind="ExternalInput")
    bp_d = nc.dram_tensor("bp", [P, 1], f32, kind="ExternalInput")
    rsqi_d = nc.dram_tensor("rsqi", [P, NL], f32, kind="ExternalInput")
    rsqo_d = nc.dram_tensor("rsqo", [P, NB], f32, kind="ExternalInput")
    src16_d = nc.dram_tensor("src16", [P, 8 * TE], i16, kind="ExternalInput")
    ind_d = nc.dram_tensor("ind8", [P, TE * P], ind_dt, kind="ExternalInput")
    indT_d = nc.dram_tensor("indT8", [P, TE * P], ind_dt, kind="ExternalInput")
    if D_FALLBACK:
        dst16_d = nc.dram_tensor("dst16", [P, 8 * TE], i16,
                                 kind="ExternalInput")
    scoresPT_d = nc.dram_tensor("scoresPT", [P, TE], f32,
                                kind="ExternalOutput")
    TBMAX = int((T_arr[:, 0] + T_arr[:, 1]).max())
    # AllGather outputs in the pair-HBM shared address space: the collective
    # takes its fast HBM-HBM path when the output tensor is Shared.
    SHARED = os.environ.get("GCN2_SHARED", "1") == "1"
    if SHARED:
        h1_fS = nc.dram_tensor("h1_fS", [NPAD, HID], bf16,
                               addr_space="Shared")
        h2_fS = nc.dram_tensor("h2_fS", [NPAD, OUT_F], bf16,
                               addr_space="Shared")
        s_fS = nc.dram_tensor("s_fS", [NPAD, P], bf16, addr_space="Shared")

    with tile.TileContext(nc) as tc:
        with (
            tc.tile_pool(name="cons", bufs=1) as cons,
            tc.tile_pool(name="sb", bufs=3) as sb,
            tc.tile_pool(name="gp",
                         bufs=int(os.environ.get("GCN2_GPBUFS", "8"))) as gp,
            tc.tile_pool(name="ps", bufs=3, space="PSUM") as ps,
            tc.tile_pool(name="psmm", bufs=2, space="PSUM") as psmm,
            tc.tile_pool(name="dram", bufs=1, space="DRAM") as dr,
        ):
            # ---- resident constants ----
            w1_sb = cons.tile([P, 2 * HID], bf16, name="w1_sb")
            w2_sb = cons.tile([P, 2 * OUT_F], bf16, name="w2_sb")
            wp_sb = cons.tile([P, 2], f32, name="wp_sb")
            b1_sb = cons.tile([P, 2], f32, name="b1_sb")
            b2_sb = cons.tile([P, 1], f32, name="b2_sb")
            bp_sb = cons.tile([P, 1], f32, name="bp_sb")
            rsqi_sb = cons.tile([P, NL], f32, name="rsqi_sb")
            rsqo_sb = cons.tile([P, NB], f32, name="rsqo_sb")
            src16 = cons.tile([P, 8 * TE], i16, name="src16")
            if D_FALLBACK:
                dst16 = cons.tile([P, 8 * TE], i16, name="dst16")
                nc.sync.dma_start(dst16[:], dst16_d[:])
            sdcol2 = cons.tile([P, 2 * NB], bf16, name="sdcol2")
            ones_sb = cons.tile([P, P], bf16, name="ones_sb")
            nc.vector.memset(ones_sb[:], 1.0)
            zfill = cons.tile([P, P], bf16, name="zfill")
            nc.vector.memset(zfill[:], 0.0)
            for s_t, d_t in [(w1_sb, w1_d), (w2_sb, w2_d), (wp_sb, wp_d),
                             (b1_sb, b1_d), (b2_sb, b2_d), (bp_sb, bp_d),
                             (rsqi_sb, rsqi_d), (rsqo_sb, rsqo_d),
                             (src16, src16_d)]:
                nc.sync.dma_start(s_t[:], d_t[:])

            # ---- DRAM intermediates ----
            # Full tables are chunk-major: rows [0, 8*R0) hold every core's
            # first AG_MID blocks (chunk A), rows [8*R0, NPAD) the rest.
            # Each chunk's AllGather output is then contiguous, so chunk A
            # can fire mid-phase and overlap the producing phase's tail.
            # The int16 gather-index split rides the same boundary.
            h1_sl = dr.tile([NL, HID], bf16, name="h1_sl")
            h2_sl = dr.tile([NL, OUT_F], bf16, name="h2_sl")
            s_sl = dr.tile([NL, P], bf16, name="s_sl")
            if SHARED:
                h1_f, h2_f, s_f = h1_fS, h2_fS, s_fS
            else:
                h1_f = dr.tile([NPAD, HID], bf16, name="h1_f")
                h2_f = dr.tile([NPAD, OUT_F], bf16, name="h2_f")
                s_f = dr.tile([NPAD, P], bf16, name="s_f")

            def ag_full(sl, f):
                nc.gpsimd.collective_compute(
                    "AllGather", mybir.AluOpType.bypass,
                    replica_groups=[list(range(NCORES))],
                    ins=[sl.opt()], outs=[f[0:NPAD, :]])

            def tab_half(f, h):
                return f[h * HALF:(h + 1) * HALF, :]

            # ---- gather helper (plain SWDGE gather, Tile-managed sync) ----
            # Emitted with queue_num=0; build_program patches queue_num to
            # (DMASW lane % NQ) after scheduling, so each SWDGE completion
            # semaphore stays bound to exactly one queue while the rings of
            # NQ queues drain in parallel.
            def do_gather_idx(dst_ap, tab_ap, idx_ap, n_idx, **kw):
                nc.gpsimd.dma_gather(
                    dst_ap, tab_ap, idx_ap,
                    n_idx, n_idx, tab_ap.ap[-1][1], **kw)

            def do_gather(dst_ap, tab_ap, idx_cols0, n_idx, **kw):
                do_gather_idx(dst_ap, tab_ap,
                              src16[:, idx_cols0:idx_cols0 + n_idx // 16],
                              n_idx, **kw)

            # ---- Phase A: h1 rows for owned nodes ----
            for b in range(NB):
                xt = sb.tile([P, 2 * P], bf16, tag="xt", name="xt")
                nc.sync.dma_start(xt[:], xT_d[:, b * 2 * P:(b + 1) * 2 * P])
                h1mm = psmm.tile([P, 4 * P], f32, tag="mm", name="h1mm")
                h1ps = h1mm[:, 0:HID]
                for c in range(2):
                    nc.tensor.matmul(
                        out=h1ps, lhsT=xt[:, c * P:(c + 1) * P],
                        rhs=w1_sb[:, c * HID:(c + 1) * HID],
                        start=(c == 0), stop=(c == 1))
                h1sb = sb.tile([P, HID], bf16, tag="h1sb", name="h1sb")
                nc.vector.tensor_copy(h1sb[:], h1ps)
                nc.sync.dma_start(h1_sl[b * P:(b + 1) * P, :], h1sb[:])
            ag_full(h1_sl, h1_f)

            if not D_FALLBACK:
                # s_sl rows beyond column 1 are never written by l2_block;
                # zero them once (overlaps phase B) so the AllGather input
                # is fully defined.
                for b in range(NB):
                    nc.sync.dma_start(s_sl[b * P:(b + 1) * P, 2:P],
                                      zfill[:, 2:P])

            def conv_phase(tab_full3, d_in, per_block_cb):
                """Gather+scatter over all (block, half) groups; one gather
                call per group. per_block_cb(b, aggT_ps_chunks) consumes the
                accumulated block aggregation ([P, P] psum chunks)."""
                nch = d_in // P
                gt0 = 0
                for b in range(NB):
                    T_blk = int(T_arr[b, 0] + T_arr[b, 1])
                    aggT_ps = [ps.tile([P, P], f32, tag=f"aggT{c}",
                                       name=f"aggT{c}") for c in range(nch)]
                    tdone = 0
                    for h in range(2):
                        T = int(T_arr[b, h])
                        tab = tab_half(tab_full3, h)
                        done = 0
                        while done < T:
                            n = min(MAX_GT, T - done)
                            t0 = gt0 + tdone + done
                            g = gp.tile([P, MAX_GT, d_in], bf16, tag="g",
                                        name="g")
                            do_gather(g[:, 0:n, :], tab, 8 * t0, P * n)
                            iw = gp.tile([P, MAX_GT * P], ind_dt, tag="iw",
                                         name="iw")
                            nc.sync.dma_start(
                                iw[:, 0:n * P],
                                ind_d[:, t0 * P:(t0 + n) * P])
                            for tt in range(n):
                                gl = tdone + done + tt
                                for c in range(nch):
                                    nc.tensor.matmul(
                                        out=aggT_ps[c][:],
                                        lhsT=g[:, tt, c * P:(c + 1) * P],
                                        rhs=iw[:, tt * P:(tt + 1) * P],
                                        start=(gl == 0),
                                        stop=(gl == T_blk - 1))
                            done += n
                        tdone += T
                    if T_blk == 0:
                        for c in range(nch):
                            nc.vector.memset(aggT_ps[c][:], 0.0)
                    per_block_cb(b, aggT_ps)
                    gt0 += T_blk

            # ---- Phase B: layer 1 + h2 rows ----
            def l1_block(b, aggT_ps):
                x1T = []
                for c in range(2):
                    z = sb.tile([P, P], f32, tag=f"z{c}", name=f"z{c}")
                    nc.vector.tensor_tensor(
                        out=z[:], in0=aggT_ps[c][:],
                        in1=rsqi_sb[:, b * P:(b + 1) * P],
                        op=mybir.AluOpType.mult)
                    xc = sb.tile([P, P], bf16, tag=f"x1T{c}", name=f"x1T{c}")
                    nc.scalar.activation(xc[:], z[:],
                                         mybir.ActivationFunctionType.Relu,
                                         bias=b1_sb[:, c:c + 1])
                    x1T.append(xc)
                h2mm = psmm.tile([P, 4 * P], f32, tag="mm", name="h2mm")
                h2ps = h2mm[:, 0:OUT_F]
                for c in range(2):
                    nc.tensor.matmul(
                        out=h2ps, lhsT=x1T[c][:],
                        rhs=w2_sb[:, c * OUT_F:(c + 1) * OUT_F],
                        start=(c == 0), stop=(c == 1))
                h2sb = sb.tile([P, OUT_F], bf16, tag="h2sb", name="h2sb")
                nc.vector.tensor_scalar(
                    out=h2sb[:], in0=h2ps, scalar1=rsqo_sb[:, b:b + 1],
                    scalar2=None, op0=mybir.AluOpType.mult)
                nc.sync.dma_start(h2_sl[b * P:(b + 1) * P, :], h2sb[:])

            if PHASE >= 2:
                conv_phase(h1_f, HID, l1_block)
                ag_full(h2_sl, h2_f)

            # ---- Phase C: layer 2 + per-node scores ----
            def l2_block(b, aggT_ps):
                z = sb.tile([P, P], f32, tag="z2", name="z2")
                nc.vector.tensor_tensor(
                    out=z[:], in0=aggT_ps[0][:],
                    in1=rsqi_sb[:, b * P:(b + 1) * P],
                    op=mybir.AluOpType.mult)
                x2T = sb.tile([P, P], f32, tag="x2T", name="x2T")
                nc.scalar.activation(x2T[:], z[:],
                                     mybir.ActivationFunctionType.Relu,
                                     bias=b2_sb[:, 0:1])
                smm = psmm.tile([P, 4 * P], f32, tag="mm", name="smm")
                sps = smm[:, 0:2]
                nc.tensor.matmul(out=sps, lhsT=x2T[:], rhs=wp_sb[:],
                                 start=True, stop=True)
                if D_FALLBACK:
                    scp = sb.tile([P, 2], f32, tag="scp", name="scp")
                    nc.vector.tensor_copy(scp[:], sps)
                    srow = sb.tile([P, P], bf16, tag="srow", name="srow")
                    nc.vector.tensor_scalar(
                        out=srow[:, 0:64], in0=ones_sb[:, 0:64],
                        scalar1=scp[:, 0:1], scalar2=None,
                        op0=mybir.AluOpType.mult)
                    nc.vector.tensor_scalar(
                        out=srow[:, 64:P], in0=ones_sb[:, 0:64],
                        scalar1=scp[:, 1:2], scalar2=None,
                        op0=mybir.AluOpType.mult)
                    nc.sync.dma_start(s_sl[b * P:(b + 1) * P, :], srow[:])
                else:
                    # DVE ops in the gather storm cost ~6.7us EACH from
                    # SBUF-port contention with Q7 desc-gen, regardless of
                    # size — one cast replaces copy+tensor_scalar+copy. The
                    # score gather only reads column 0 of each table row, so
                    # s_sl rows get just [s_src, s_dst] (rest stays zero).
                    nc.vector.tensor_copy(sdcol2[:, 2 * b:2 * b + 2], sps)
                    nc.sync.dma_start(s_sl[b * P:(b + 1) * P, 0:2],
                                      sdcol2[:, 2 * b:2 * b + 2])

            if PHASE >= 3:
                conv_phase(h2_f, OUT_F, l2_block)
                ag_full(s_sl, s_f)

            # ---- Phase D: per-edge scores ----
            # s_src[src[e]] via one gather of s_f rows ([edge-slot partition,
            # tile, col] layout, col 0 holds the value). s_dst[dst[e]] needs
            # NO gather: per tile, matmul(lhsT=indT tile [dstcol, slot],
            # rhs=sdcol[:, b]) permutes the block's s_dst column into slot
            # order. (In fallback mode s_dst is gathered from s_sl instead.)
            gt0 = 0
            for b in (range(NB) if PHASE >= 4 else []):
                T_blk = int(T_arr[b, 0] + T_arr[b, 1])
                if T_blk == 0:
                    continue
                gA = gp.tile([P, TBMAX, P], bf16, tag="gA", name="gA")
                if D_FALLBACK:
                    gB = gp.tile([P, TBMAX, P], bf16, tag="gB", name="gB")
                else:
                    sd_ps = ps.tile([P, TBMAX], f32, tag="aggT0", name="sd")
                tdone = 0
                for h in range(2):
                    T = int(T_arr[b, h])
                    tab = tab_half(s_f, h)
                    done = 0
                    while done < T:
                        n = min(MAX_GT, T - done)
                        t0 = gt0 + tdone + done
                        sl = tdone + done
                        do_gather(gA[:, sl:sl + n, :], tab, 8 * t0, P * n)
                        if not D_FALLBACK:
                            iw = gp.tile([P, MAX_GT * P], ind_dt, tag="iwT",
                                         name="iwT")
                            nc.sync.dma_start(
                                iw[:, 0:n * P],
                                indT_d[:, t0 * P:(t0 + n) * P])
                            for tt in range(n):
                                nc.tensor.matmul(
                                    out=sd_ps[:, sl + tt:sl + tt + 1],
                                    lhsT=iw[:, tt * P:(tt + 1) * P],
                                    rhs=sdcol2[:, 2 * b + 1:2 * b + 2],
                                    start=True, stop=True)
                        done += n
                    tdone += T
                if D_FALLBACK:
                    done = 0
                    while done < T_blk:
                        n = min(MAX_GT, T_blk - done)
                        t0 = gt0 + done
                        do_gather_idx(gB[:, done:done + n, :], s_sl[:],
                                      dst16[:, 8 * t0:8 * (t0 + n)], P * n)
                        done += n
                    sd_in = gB[:, 0:T_blk, 64]
                else:
                    sd_in = sd_ps[:, 0:T_blk]
                lsum = sb.tile([P, TBMAX], f32, tag="lsum", name="lsum")
                nc.vector.tensor_tensor(
                    out=lsum[:, 0:T_blk], in0=gA[:, 0:T_blk, 0], in1=sd_in,
                    op=mybir.AluOpType.add)
                scf = sb.tile([P, TBMAX], f32, tag="scf", name="scf")
                nc.scalar.activation(scf[:, 0:T_blk], lsum[:, 0:T_blk],
                                     mybir.ActivationFunctionType.Sigmoid,
                                     bias=bp_sb[:, 0:1])
                nc.sync.dma_start(
                    scoresPT_d[:, gt0:gt0 + T_blk], scf[:, 0:T_blk])
                gt0 += T_blk

    # Post-schedule queue spread: Tile assigned every Pool-engine DMA a
    # DMASW lane (round-robin mod 8) and its completion sem comes from that
    # lane, so queue_num = lane % NQ keeps each sem updated from exactly one
    # queue (the hardware bookkeeping requirement) while NQ queues' rings
    # drain concurrently.
    if NQ > 1:
        import bass_rust as _br
        dmasw0 = _br.PROC_NAMES.index("DMASW0")
        n_patch = 0
        other_lanes = set()
        for f in nc.m.functions:
            for bb in f.blocks:
                for inst in bb.instructions:
                    proc = getattr(inst, "bass_scheduled_proc", None)
                    if proc is None or not (dmasw0 <= proc < dmasw0 + 8):
                        continue
                    if isinstance(inst, mybir.InstDMAGatherAnt):
                        # lane % NQ is the only mapping validated on HW;
                        # lane//2 (pairing lanes onto one queue) hangs.
                        lane = proc - dmasw0
                        if os.environ.get("GCN2_QMAP", "mod") == "pair":
                            inst.queue_num = (lane // 2) % NQ
                        else:
                            inst.queue_num = lane % NQ
                        n_patch += 1
                    else:
                        other_lanes.add(proc - dmasw0)
        # non-gather SWDGE DMAs stay on queue 0; their lanes must map to
        # queue 0 under the same rule or the sem/queue binding breaks.
        assert not other_lanes, (
            f"non-gather SWDGE DMAs on lanes {sorted(other_lanes)} — "
            f"queue mapping must send these lanes to queue 0")
        assert n_patch > 0

    nc.compile()
    return nc


def preprocess(features, src, dst, W1, b1, W2, b2, Wp, bp):
    """Sort/pad edges, build per-core input maps + reassembly info."""
    E = src.shape[0]
    src = src.astype(np.int64)
    dst = dst.astype(np.int64)
    n_nodes = features.shape[0]

    deg_out = np.bincount(src, minlength=n_nodes).astype(np.float64)
    deg_in = np.bincount(dst, minlength=n_nodes).astype(np.float64)
    rsq_out = (1.0 / np.sqrt(np.clip(deg_out, 1.0, None))).astype(np.float32)
    rsq_in = (1.0 / np.sqrt(np.clip(deg_in, 1.0, None))).astype(np.float32)

    NB = BPC
    NL = NB * P
    # edge sort: (dst block, src half, ...)
    gblk = dst // P
    half = (src >= HALF).astype(np.int64)
    key = gblk * 2 + half
    order = np.argsort(key, kind="stable")
    key_s = key[order]
    bounds = np.searchsorted(key_s, np.arange(2 * (NPAD // P) + 1))
    cnt = np.diff(bounds).reshape(NCORES, NB, 2)
    T_arr = np.ceil(cnt.max(axis=0) / P).astype(np.int64)   # [NB, 2]
    TE = int(T_arr.sum())

    src_s = src[order]
    dst_s = dst[order]

    # slot offset of each (b, h) group
    goff = np.zeros((NB, 2), np.int64)
    off = 0
    for b in range(NB):
        for h in range(2):
            goff[b, h] = off
            off += int(T_arr[b, h]) * P
    assert off == TE * P

    slot_src = np.zeros((NCORES, TE * P), np.int64)
    slot_col = np.full((NCORES, TE * P), -1, np.int64)   # -1 = pad
    slot_orig = np.full((NCORES, TE * P), -1, np.int64)

    for core in range(NCORES):
        for b in range(NB):
            g = core * NB + b
            for h in (0, 1):
                lo_e, hi_e = bounds[2 * g + h], bounds[2 * g + h + 1]
                nn = hi_e - lo_e
                o = goff[b, h]
                slot_src[core, o:o + nn] = src_s[lo_e:hi_e]
                slot_col[core, o:o + nn] = dst_s[lo_e:hi_e] - (core * NL + b * P)
                slot_orig[core, o:o + nn] = order[lo_e:hi_e]

    # host-side tensors
    xpad = np.zeros((NPAD, IN_F), np.float32)
    xpad[:n_nodes] = features * rsq_out[:, None]
    rsqi_pad = np.zeros(NPAD, np.float32)
    rsqi_pad[:n_nodes] = rsq_in
    rsqo_pad = np.zeros(NPAD, np.float32)
    rsqo_pad[:n_nodes] = rsq_out

    w1h = np.concatenate([W1[:P, :], W1[P:, :]], axis=1).astype(ml_dtypes.bfloat16)
    w2h = np.concatenate([W2[:P, :], W2[P:, :]], axis=1).astype(ml_dtypes.bfloat16)
    wp12 = np.stack([Wp[:OUT_F, 0], Wp[OUT_F:, 0]], axis=1).astype(np.float32)
    b1c = np.stack([b1[:P], b1[P:]], axis=1).astype(np.float32)
    b2c = b2.astype(np.float32)[:, None]
    bpc = np.full((P, 1), np.float32(bp[0]))

    lane = np.arange(TE * P) % P
    tidx = np.arange(TE * P) // P

    in_maps = []
    for core in range(NCORES):
        base = core * NL
        xT = np.zeros((P, NB * 2 * P), ml_dtypes.bfloat16)
        for b in range(NB):
            blk = xpad[base + b * P: base + (b + 1) * P]   # [128, 256]
            for c in range(2):
                xT[:, (b * 2 + c) * P:(b * 2 + c + 1) * P] = \
                    blk[:, c * P:(c + 1) * P].T.astype(ml_dtypes.bfloat16)
        rsqi_rep = np.broadcast_to(rsqi_pad[base:base + NL][None, :],
                                   (P, NL)).astype(np.float32).copy()
        rsqo_col = rsqo_pad[base:base + NL].reshape(NB, P).T.copy()

        scol = slot_col[core]
        valid = scol >= 0
        hdt = ml_dtypes.bfloat16 if IND_BF16 else ml_dtypes.float8_e4m3
        ind8 = np.zeros((P, TE * P), hdt)
        indT8 = np.zeros((P, TE * P), hdt)
        ind8[lane[valid], tidx[valid] * P + scol[valid]] = 1.0
        indT8[scol[valid], tidx[valid] * P + lane[valid]] = 1.0

        ssrc = slot_src[core]
        s16 = np.where(ssrc >= HALF, ssrc - HALF, ssrc)
        m = dict(
            xT=xT, w1=w1h, w2=w2h, wp12=wp12, b1c=b1c, b2c=b2c, bp=bpc,
            rsqi=rsqi_rep, rsqo=rsqo_col,
            src16=_wrap_idx(s16),
            ind8=ind8, indT8=indT8,
        )
        if D_FALLBACK:
            # local dst node index per slot (block * 128 + col); pads -> 0
            blk_of_slot = np.zeros(TE * P, np.int64)
            off2 = 0
            for b in range(NB):
                w2_ = int(T_arr[b].sum()) * P
                blk_of_slot[off2:off2 + w2_] = b
                off2 += w2_
            dloc = np.where(valid, blk_of_slot * P + np.maximum(scol, 0), 0)
            m["dst16"] = _wrap_idx(dloc)
        in_maps.append(m)

    return in_maps, slot_orig, T_arr, E


_CACHE = {}


def _get_program(T_arr):
    key = (tuple(map(tuple, T_arr)), os.environ.get("GCN2_PHASE", "4"),
           os.environ.get("GCN2_MAXGT"), os.environ.get("GCN2_AG_MID"),
           os.environ.get("GCN2_SHARED"), os.environ.get("GCN2_GPBUFS"),
           IND_BF16, D_FALLBACK, NQ, SCRATCH)
    if key not in _CACHE:
        _CACHE[key] = build_program(np.asarray(T_arr))
    return _CACHE[key]


def kernel(features, src, dst, edge_type, W1, b1, W2, b2, Wp, bp, _trace=False,
           _tmpdir=None):
    features = np.asarray(features, np.float32)
    src_i = np.asarray(src, np.int32)
    dst_i = np.asarray(dst, np.int32)
    in_maps, slot_orig, T_arr, E = preprocess(
        features, src_i, dst_i, np.asarray(W1), np.asarray(b1),
        np.asarray(W2), np.asarray(b2), np.asarray(Wp), np.asarray(bp))
    nc = _get_program(T_arr)
    res = run_bass_kernel_spmd(nc, in_maps, core_ids=list(range(NCORES)),
                               trace=_trace, tmpdir=_tmpdir)
    out = np.zeros(E, np.float32)
    for core in range(NCORES):
        sc = np.asarray(res.results[core]["scoresPT"])   # [P, TE]
        flat = sc.T.reshape(-1)
        so = slot_orig[core]
        m = so >= 0
        out[so[m]] = flat[m[:flat.shape[0]] if m.shape[0] > flat.shape[0] else m]
    if _trace:
        kernel._last_results = res
    return out
